# revision 3
# baseline (speedup 1.0000x reference)
"""Trainium2 Bass kernel for nn_AdaptiveFullConnected (segment_reduce).

Reference computation (per batch b):
    c      = coords + depthwise_conv1d(coords, K=5) + conv_b          [N, 2]
    h      = gelu(c @ lin1_w.T + lin1_b)                              [N, 512]
    weight = h @ lin2_w.T + lin2_b                                    [N, 512]
    xw     = tile(x, 8) * weight                                      [N, 512]
    mean_p = mean over {n : idx[n] == p} of xw[n, :]                  [P, 512]
    out    = w1 * sin(mean) + w2 * cos(mean)                          [P, 512]

Sharding: 8 cores = (batch b = core//2) x (half of N = core%2), 8192 rows
per core.  Each core computes partial segment sums for all 256 segments as
a one-hot matmul, a pairwise ReduceScatter combines the two halves (core
2b keeps segments 0:128, core 2b+1 keeps 128:256), and the epilogue
(bias-fold, mean, sin/cos) runs on the 128 rows each core owns.

The lin2 bias is folded through the segment reduce:
    seg(x * (w_nb + b2)) = seg(x * w_nb) + b2 * seg(x)
so the device never materializes a bias add over [N, 512]; instead the
segment matmul carries 577 columns: 512 for x*w_nb, 64 for seg(x) (x has
only 64 unique columns), 1 for the segment counts.
"""

import numpy as np
from contextlib import ExitStack

B = 4
N = 16384
DIMS = 64
HEADS = 8
D = DIMS * HEADS  # 512
K = 5
PFULL = 256
NCORES = 8
NLOC = N // 2  # 8192 rows per core
NT = NLOC // 128  # 64 k-tiles
CHUNK = 512  # n-chunk for lin1/lin2
NCH = NLOC // CHUNK  # 16
ET = D // 128  # 4 e-tiles
SEGW = D + DIMS + 1  # 577
GROUPS = [[0, 1], [2, 3], [4, 5], [6, 7]]

_CACHE = {}


def build_nc():
    import concourse.bass as bass  # noqa: F401
    import concourse.mybir as mybir
    import concourse.tile as tile
    from concourse import bacc

    f16 = mybir.dt.float16
    f32 = mybir.dt.float32
    i32 = mybir.dt.int32
    mult = mybir.AluOpType.mult
    add = mybir.AluOpType.add
    is_equal = mybir.AluOpType.is_equal
    AF = mybir.ActivationFunctionType

    nc = bacc.Bacc("TRN2", num_devices=NCORES)

    x16 = nc.declare_dram_parameter("x16", [128, NT * DIMS], f16, isOutput=False)
    idxs = nc.declare_dram_parameter("idxs", [128, NT], i32, isOutput=False)
    conv_in = nc.declare_dram_parameter("conv_in", [128, 132], f32, isOutput=False)
    w1aug = nc.declare_dram_parameter("w1aug", [3, D], f16, isOutput=False)
    w2t = nc.declare_dram_parameter("w2t", [128, ET * D], f16, isOutput=False)
    b2rep = nc.declare_dram_parameter("b2rep", [128, D], f32, isOutput=False)
    consts = nc.declare_dram_parameter("consts", [128, 16], f32, isOutput=False)
    onesrow = nc.declare_dram_parameter("onesrow", [1, NLOC], f16, isOutput=False)
    out = nc.declare_dram_parameter("out", [128, D], f32, isOutput=True)

    with tile.TileContext(nc, num_cores=NCORES) as tc, ExitStack() as ctx:
        cpool = ctx.enter_context(tc.tile_pool(name="cpool", bufs=1))
        work = ctx.enter_context(tc.tile_pool(name="work", bufs=1))
        psum = ctx.enter_context(tc.tile_pool(name="psum", bufs=1, space="PSUM"))
        dram = ctx.enter_context(tc.tile_pool(name="dram", bufs=1, space="DRAM"))

        # ---- constant loads ----
        x_sb = cpool.tile([128, NT, DIMS], f16)
        nc.sync.dma_start(out=x_sb[:], in_=x16[:].rearrange("p (t c) -> p t c", c=DIMS))
        idx_sb = cpool.tile([128, NT], i32)
        nc.sync.dma_start(out=idx_sb[:], in_=idxs[:])
        ci_sb = cpool.tile([128, 132], f32)
        nc.sync.dma_start(out=ci_sb[:], in_=conv_in[:])
        w1_sb = cpool.tile([3, D], f16)
        nc.sync.dma_start(out=w1_sb[:], in_=w1aug[:])
        w2_sb = cpool.tile([128, ET, D], f16)
        nc.sync.dma_start(out=w2_sb[:], in_=w2t[:].rearrange("p (e d) -> p e d", d=D))
        b2_sb = cpool.tile([128, D], f32)
        nc.sync.dma_start(out=b2_sb[:], in_=b2rep[:])
        cst = cpool.tile([128, 16], f32)
        nc.sync.dma_start(out=cst[:], in_=consts[:])
        iota_sb = cpool.tile([128, PFULL], i32)
        nc.gpsimd.iota(iota_sb[:], pattern=[[1, PFULL]], base=0, channel_multiplier=0)

        # ---- depthwise conv on coords ----
        # ci_sb row p = (ch, r): ci_sb[p, j] = coords_pad[r*128 + j, ch]
        # local n = r*128 + jj :  c[n] = ci[jj+2] + conv_b + sum_k w_k * ci[jj+k]
        acc0 = work.tile([128, 128], f32, name="acc0")
        acc1 = work.tile([128, 128], f32, name="acc1")
        nc.vector.tensor_scalar(
            out=acc0[:], in0=ci_sb[:, 0:128], scalar1=cst[:, 0:1], scalar2=None, op0=mult
        )
        accs = [acc0, acc1]
        for k in range(1, K):
            src, dst = accs[(k + 1) % 2], accs[k % 2]
            nc.vector.scalar_tensor_tensor(
                out=dst[:], in0=ci_sb[:, k : k + 128], scalar=cst[:, k : k + 1],
                in1=src[:], op0=mult, op1=add,
            )
        # after k=4 the live accumulator is accs[0]
        cfin = work.tile([128, 128], f16, name="cfin")
        nc.vector.scalar_tensor_tensor(
            out=cfin[:], in0=ci_sb[:, 2:130], scalar=cst[:, 5:6], in1=accs[0][:],
            op0=add, op1=add,
        )
        # shuffle [128=(ch,r), 128] -> [2, 8192] via DRAM bounce
        cbounce = dram.tile([2, NLOC], f16)
        nc.sync.dma_start(
            out=cbounce[:].rearrange("c (r j) -> (c r) j", j=128), in_=cfin[:]
        )
        cT = cpool.tile([3, NLOC], f16)
        nc.sync.dma_start(out=cT[0:2, :], in_=cbounce[:])
        nc.sync.dma_start(out=cT[2:3, :], in_=onesrow[:])

        # ---- persistent segment accumulators (PSUM, 2 banks each) ----
        pseg = [psum.tile([128, SEGW], f32, name=f"pseg{i}") for i in range(2)]

        # ---- main loop ----
        for c in range(NCH):
            hts = []
            for e in range(ET):
                ph = psum.tile([128, CHUNK], f32, name="ph", bufs=2)
                nc.tensor.matmul(
                    ph[:],
                    lhsT=w1_sb[:, e * 128 : (e + 1) * 128],
                    rhs=cT[:, c * CHUNK : (c + 1) * CHUNK],
                    start=True, stop=True,
                )
                ht = work.tile([128, CHUNK], f16, name=f"ht{e}", bufs=2)
                nc.scalar.activation(out=ht[:], in_=ph[:], func=AF.Gelu)
                hts.append(ht)
            for t4 in range(CHUNK // 128):
                kt = c * (CHUNK // 128) + t4
                pw = psum.tile([128, D], f32, name="pw", bufs=2)
                for e in range(ET):
                    nc.tensor.matmul(
                        pw[:],
                        lhsT=hts[e][:, t4 * 128 : (t4 + 1) * 128],
                        rhs=w2_sb[:, e, :],
                        start=(e == 0), stop=(e == ET - 1),
                    )
                xwa = work.tile([128, SEGW], f16, name="xwa", bufs=3)
                xv = x_sb[:, kt, :].unsqueeze(1).to_broadcast([128, HEADS, DIMS])
                nc.vector.tensor_tensor(
                    out=xwa[:, 0:D].rearrange("p (h c) -> p h c", c=DIMS),
                    in0=pw[:].rearrange("p (h c) -> p h c", c=DIMS),
                    in1=xv, op=mult,
                )
                nc.vector.tensor_copy(out=xwa[:, D : D + DIMS], in_=x_sb[:, kt, :])
                nc.vector.memset(xwa[:, D + DIMS : SEGW], 1.0)
                oh = work.tile([128, PFULL], f16, name="oh", bufs=3)
                nc.vector.tensor_tensor(
                    out=oh[:],
                    in0=idx_sb[:, kt : kt + 1].to_broadcast([128, PFULL]),
                    in1=iota_sb[:], op=is_equal,
                )
                for p2 in range(2):
                    lhs = oh[:, p2 * 128 : (p2 + 1) * 128]
                    nc.tensor.matmul(
                        pseg[p2][:, 0:D], lhsT=lhs, rhs=xwa[:, 0:D],
                        start=(kt == 0), stop=(kt == NT - 1),
                    )
                    nc.tensor.matmul(
                        pseg[p2][:, D:SEGW], lhsT=lhs, rhs=xwa[:, D:SEGW],
                        start=(kt == 0), stop=(kt == NT - 1),
                    )

        # ---- drain partials, pairwise reduce-scatter ----
        seg_part = dram.tile([PFULL, SEGW], f32)
        for p2 in range(2):
            s = work.tile([128, SEGW], f32, name=f"seg_sb{p2}")
            nc.vector.tensor_copy(out=s[:], in_=pseg[p2][:])
            nc.sync.dma_start(out=seg_part[p2 * 128 : (p2 + 1) * 128, :], in_=s[:])
        seg_red = dram.tile([128, SEGW], f32)
        nc.gpsimd.collective_compute(
            "ReduceScatter",
            mybir.AluOpType.add,
            replica_groups=GROUPS,
            ins=[seg_part[:]],
            outs=[seg_red[:]],
        )

        # ---- epilogue on the 128 owned segments ----
        red = work.tile([128, SEGW], f32, name="red")
        nc.sync.dma_start(out=red[:], in_=seg_red[:])
        rec = work.tile([128, 1], f32, name="rec")
        nc.vector.reciprocal(out=rec[:], in_=red[:, SEGW - 1 : SEGW])
        t1 = work.tile([128, D], f32, name="t1")
        xsegv = red[:, D : D + DIMS].unsqueeze(1).to_broadcast([128, HEADS, DIMS])
        nc.vector.tensor_tensor(
            out=t1[:].rearrange("p (h c) -> p h c", c=DIMS),
            in0=b2_sb[:].rearrange("p (h c) -> p h c", c=DIMS),
            in1=xsegv, op=mult,
        )
        t2 = work.tile([128, D], f32, name="t2")
        nc.vector.tensor_tensor(out=t2[:], in0=t1[:], in1=red[:, 0:D], op=add)
        mean = work.tile([128, D], f32, name="mean")
        nc.vector.tensor_scalar(
            out=mean[:], in0=t2[:], scalar1=rec[:, 0:1], scalar2=None, op0=mult
        )
        sinp = work.tile([128, D], f32, name="sinp")
        nc.scalar.activation(out=sinp[:], in_=mean[:], func=AF.Sin)
        cosp = work.tile([128, D], f32, name="cosp")
        nc.scalar.activation(out=cosp[:], in_=mean[:], func=AF.Sin, bias=cst[:, 6:7])
        sins = work.tile([128, D], f32, name="sins")
        nc.vector.tensor_scalar(
            out=sins[:], in0=sinp[:], scalar1=cst[:, 7:8], scalar2=None, op0=mult
        )
        out_sb = work.tile([128, D], f32, name="out_sb")
        nc.vector.scalar_tensor_tensor(
            out=out_sb[:], in0=cosp[:], scalar=cst[:, 8:9], in1=sins[:],
            op0=mult, op1=add,
        )
        nc.sync.dma_start(out=out[:], in_=out_sb[:])

    nc.finalize()
    return nc


def make_in_maps(x, coords, indices, conv_w, conv_b, lin1_w, lin1_b, lin2_w,
                 lin2_b, w1, w2):
    """Host-side sharding + layout prep.  Returns list of 8 input dicts."""
    x = np.asarray(x, np.float32)
    coords = np.asarray(coords, np.float32)
    idx_full = np.asarray(indices).reshape(B, N).astype(np.int32)
    conv_w = np.asarray(conv_w, np.float32)
    conv_b = np.asarray(conv_b, np.float32)
    lin1_w = np.asarray(lin1_w, np.float32)
    lin1_b = np.asarray(lin1_b, np.float32)
    lin2_w = np.asarray(lin2_w, np.float32)
    lin2_b = np.asarray(lin2_b, np.float32)

    w1aug = np.stack([lin1_w[:, 0], lin1_w[:, 1], lin1_b]).astype(np.float16)  # [3, D]
    w2t = np.ascontiguousarray(lin2_w.T)  # [e, d]
    w2t_sh = (
        w2t.reshape(ET, 128, D).transpose(1, 0, 2).reshape(128, ET * D)
        .astype(np.float16)
    )
    b2rep = np.tile(lin2_b[None, :], (128, 1)).astype(np.float32)
    consts = np.zeros((128, 16), np.float32)
    ch_of_p = (np.arange(128) // 64)  # 0 for rows 0:64, 1 for 64:128
    for k in range(K):
        consts[:, k] = conv_w[ch_of_p, 0, k]
    consts[:, 5] = conv_b[ch_of_p]
    consts[:, 6] = np.pi / 2
    consts[:, 7] = np.float32(np.asarray(w1).reshape(-1)[0])
    consts[:, 8] = np.float32(np.asarray(w2).reshape(-1)[0])
    onesrow = np.ones((1, NLOC), np.float16)

    r = np.arange(64)
    j = np.arange(132)
    halo_idx = r[:, None] * 128 + j[None, :]  # [64, 132] indices into coords_pad

    in_maps = []
    for core in range(NCORES):
        b, half = core // 2, core % 2
        lo = half * NLOC
        xs = x[b, lo : lo + NLOC, :]  # [8192, 64]
        x_sh = (
            xs.reshape(NT, 128, DIMS).transpose(1, 0, 2).reshape(128, NT * DIMS)
            .astype(np.float16)
        )
        idx_sh = np.ascontiguousarray(
            idx_full[b, lo : lo + NLOC].reshape(NT, 128).T
        ).astype(np.int32)
        cpad = np.zeros((NLOC + 4, 2), np.float32)
        glo, ghi = lo - 2, lo + NLOC + 2
        slo, shi = max(glo, 0), min(ghi, N)
        cpad[slo - glo : shi - glo] = coords[b, slo:shi, :]
        conv_in = np.concatenate(
            [cpad[halo_idx, 0], cpad[halo_idx, 1]], axis=0
        ).astype(np.float32)  # [128, 132]
        in_maps.append(
            dict(
                x16=x_sh, idxs=idx_sh, conv_in=conv_in, w1aug=w1aug, w2t=w2t_sh,
                b2rep=b2rep, consts=consts, onesrow=onesrow,
            )
        )
    return in_maps


def assemble(results):
    """[8 x {'out': [128, 512]}] -> [B, PFULL, D] float32."""
    out = np.empty((B, PFULL, D), np.float32)
    for core in range(NCORES):
        b, half = core // 2, core % 2
        out[b, half * 128 : (half + 1) * 128, :] = results[core]["out"]
    return out


def kernel(x, coords, indices, patch_seq_len, conv_w, conv_b, lin1_w, lin1_b,
           lin2_w, lin2_b, w1, w2):
    from concourse.bass_utils import run_bass_kernel_spmd

    if "nc" not in _CACHE:
        _CACHE["nc"] = build_nc()
    nc = _CACHE["nc"]
    in_maps = make_in_maps(x, coords, indices, conv_w, conv_b, lin1_w, lin1_b,
                           lin2_w, lin2_b, w1, w2)
    res = run_bass_kernel_spmd(nc, in_maps, core_ids=list(range(NCORES)))
    return assemble(res.results)
